# revision 12
# baseline (speedup 1.0000x reference)
"""Trainium2 Bass kernel for 2-layer GATv2 + output projection (SPMD, 8 cores).

Strategy: nodes partitioned across cores; edges sorted by destination and
packed into fixed 128-node windows (J=5 tiles of 512 edge slots each, window
addressing static so the SPMD program is uniform). Per tile: edge features +
xr[dst] accumulate in PSUM via matmuls (selector matrices built on-device),
gathered xl[src] (dma_gather, two int16-half calls) transposed into the same
PSUM; leaky-relu; per-head logits via PE; exp on ACT; segment softmax
denominators + weighted scatter-add via selector matmuls accumulated per
window. Halo exchange = AllGather of xl shards. fp32r (TF32-like) matmuls
with host/DVE-rounded operands; fp32 accumulate.
"""
import numpy as np

import concourse.bass as bass
import concourse.bacc as bacc
import concourse.mybir as mybir
import concourse.tile as tile
from concourse.bass_utils import run_bass_kernel_spmd
from concourse.masks import make_identity

F32 = mybir.dt.float32
F32R = mybir.dt.float32r
I16 = mybir.dt.int16
I32 = mybir.dt.int32

NEG_SLOPE = 0.2
EPS = 1e-30


def f32r_round(x):
    b = np.ascontiguousarray(x, np.float32).view(np.uint32)
    q = (b + 0x7FF + ((b >> 12) & 1)) & np.uint32(0xFFFFF000)
    return q.view(np.float32)


class Cfg:
    def __init__(self, N, E, IN_F, NC, J):
        self.N, self.E, self.IN_F, self.NC, self.J = N, E, IN_F, NC, J
        self.F = 256
        self.H, self.C = 4, 64
        self.W = 128                      # nodes per window
        assert N % NC == 0
        self.NV = N // NC                 # nodes per core
        self.NW = (self.NV + self.W - 1) // self.W
        self.NVP = self.NW * self.W       # padded nodes per core
        self.T_E = 512                    # edge slots per tile
        self.G = 4                        # 128-edge groups per tile
        self.SW = self.J * self.T_E       # edge slots per window
        self.NT = self.NW * self.J        # tiles per core
        self.NFULL = self.NC * self.NVP   # rows in gathered xl table
        assert self.NFULL % 2 == 0
        self.HALF = self.NFULL // 2
        assert self.HALF <= 32768, "int16 gather index range"
        self.KCH = self.IN_F // 128       # K-chunks for layer-0 projection
        import os
        _ph = os.environ.get("K_PHASES", "p1,ag0,e0,p4,ag1,e1")
        self.phases = tuple(x for x in _ph.split(",") if x)


def preprocess(cfg, edge_index, edge_attr):
    """Sort edges by dst, pack into windows/tiles, build per-core arrays."""
    src = np.asarray(edge_index[0], np.int64)
    dst = np.asarray(edge_index[1], np.int64)
    ea = np.asarray(edge_attr, np.float32)
    NV, NVP, W, NW, J, SW, T_E, NT = (cfg.NV, cfg.NVP, cfg.W, cfg.NW, cfg.J,
                                      cfg.SW, cfg.T_E, cfg.NT)
    # padded-global row of each node's xl entry
    core_of = src // NV
    grow = core_of * NVP + (src - core_of * NV)

    order = np.argsort(dst, kind="stable")
    cores = []
    for c in range(cfg.NC):
        lo = np.searchsorted(dst, c * NV, side="left", sorter=order)
        hi = np.searchsorted(dst, (c + 1) * NV, side="left", sorter=order)
        eidx_c = order[lo:hi]
        dloc = dst[eidx_c] - c * NV
        win = dloc // W

        TC = T_E - 1                      # real-edge capacity per tile
        idx_hi = np.zeros((NT, T_E), np.int16)
        idx_lo = np.full((NT, T_E), -1, np.int16)
        klow = np.zeros(NT, np.int32)
        dstf = np.full((NT, T_E), -1.0, np.float32)
        eat = np.zeros((NT, T_E, ea.shape[1]), np.float32)

        for w in range(NW):
            e_w = eidx_c[win == w]
            g_w = grow[e_w]
            o = np.argsort(g_w, kind="stable")
            e_w, g_w = e_w[o], g_w[o]
            n_e = len(e_w)
            assert n_e <= J * TC, f"window overflow: {n_e} > {J * TC}; raise J"
            for j in range(J):
                t = w * J + j
                ec = e_w[j * TC:(j + 1) * TC]
                gc = g_w[j * TC:(j + 1) * TC]
                ne = len(ec)
                kl = int((gc < cfg.HALF).sum()) + 1   # + leading dummy slot
                idx_lo[t, 0] = 0
                idx_lo[t, 1:kl] = gc[:kl - 1].astype(np.int16)
                idx_hi[t, kl:kl + (ne - kl + 1)] = (gc[kl - 1:] - cfg.HALF
                                                    ).astype(np.int16)
                klow[t] = kl
                if ne:
                    dstf[t, 1:ne + 1] = (dst[ec] - c * NV - w * W
                                         ).astype(np.float32)
                    eat[t, 1:ne + 1] = ea[ec]

        wrap = lambda a: np.tile(a.reshape(NT, T_E // 16, 16).transpose(0, 2, 1),
                                 (1, 8, 1)).copy()
        # per-tile layouts
        dstf_t = dstf.reshape(NT, 4, 128).transpose(0, 2, 1).copy()
        dstrow = dstf.copy()                        # [NT, 512]
        eat_t = f32r_round(np.ascontiguousarray(eat.transpose(0, 2, 1)))
        cores.append(dict(idx_hi=wrap(idx_hi), idx_lo=wrap(idx_lo),
                          klow=klow.reshape(1, NT), dstf=dstf_t,
                          dstrow=dstrow, eat=eat_t))
    return cores


def build_program(cfg, nc):
    """Emit the full SPMD program into nc (a Bacc) under TileContext."""
    F, G, T_E, J, NW, NT, NVP, W = (cfg.F, cfg.G, cfg.T_E, cfg.J, cfg.NW,
                                    cfg.NT, cfg.NVP, cfg.W)
    EF = 32
    # ---- external inputs
    P = {}
    def inp(name, shape, dt):
        P[name] = nc.dram_tensor(name, shape, dt, kind="ExternalInput")
        return P[name]

    x_T = inp("x_T", [cfg.IN_F, NVP], F32R)
    idx_hi = inp("idx_hi", [NT, 128, T_E // 16], I16)
    idx_lo = inp("idx_lo", [NT, 128, T_E // 16], I16)
    klow = inp("klow", [1, NT], I32)
    dstf = inp("dstf", [NT, 128, G], F32)
    dstrow = inp("dstrow", [NT, T_E], F32)
    eat = inp("eat", [NT, EF, T_E], F32R)
    wl0 = inp("wl0", [cfg.IN_F, F], F32R)
    wr0 = inp("wr0", [cfg.IN_F, F], F32R)
    we0 = inp("we0", [EF, F], F32R)
    wl1 = inp("wl1", [F, F], F32R)
    wr1 = inp("wr1", [F, F], F32R)
    we1 = inp("we1", [EF, F], F32R)
    wout = inp("wout", [F, 1], F32)
    att0 = inp("att0", [128, 2, 4], F32)
    att1 = inp("att1", [128, 2, 4], F32)
    bl0 = inp("bl0", [128, F], F32)
    br0 = inp("br0", [128, F], F32)
    bias0 = inp("bias0", [128, F], F32)
    bl1 = inp("bl1", [128, F], F32)
    br1 = inp("br1", [128, F], F32)
    bias1 = inp("bias1", [128, F], F32)
    bout = inp("bout", [128, 1], F32)
    iota_r = inp("iota_r", [128, 128], F32)
    iota_c = inp("iota_c", [128, 1], F32)

    out_own = nc.dram_tensor("out_own", [NVP, 1], F32, kind="ExternalOutput")

    # ---- internal DRAM
    xl0_own = nc.dram_tensor("xl0_own", [NVP, F], F32R)
    xr0_own = nc.dram_tensor("xr0_own", [NVP, F], F32R)
    xl1_own = nc.dram_tensor("xl1_own", [NVP, F], F32R)
    xr1_own = nc.dram_tensor("xr1_own", [NVP, F], F32R)
    akw = dict(addr_space="Shared") if cfg.NC > 4 else {}
    xl0_full = nc.dram_tensor("xl0_full", [cfg.NFULL, F], F32R, **akw)
    xl1_full = nc.dram_tensor("xl1_full", [cfg.NFULL, F], F32R, **akw)
    h1_own = nc.dram_tensor("h1_own", [NVP, F], F32)
    groups = [list(range(cfg.NC))]

    with tile.TileContext(nc) as tc:
        with (
            tc.tile_pool(name="const", bufs=1) as constp,
            tc.tile_pool(name="wpool", bufs=1) as wpool,
            tc.tile_pool(name="io", bufs=4) as io,
            tc.tile_pool(name="sel", bufs=2 * J) as selp,
            tc.tile_pool(name="mpool", bufs=2) as mpool,
            tc.tile_pool(name="small", bufs=4) as small,
            tc.tile_pool(name="psA", bufs=2, space="PSUM") as psA,
            tc.tile_pool(name="psW", bufs=2, space="PSUM") as psW,
            tc.tile_pool(name="psS", bufs=2, space="PSUM") as psS,
        ):
            ident = constp.tile([128, 128], F32)
            make_identity(nc, ident[:])
            iota_row = constp.tile([128, 128], F32)
            nc.sync.dma_start(out=iota_row[:], in_=iota_r[:])
            iota_col = constp.tile([128, 1], F32)
            nc.sync.dma_start(out=iota_col[:], in_=iota_c[:])
            klow_sb = constp.tile([1, NT], I32)
            nc.sync.dma_start(out=klow_sb[:], in_=klow[:])
            batt = {}
            for nm, t in (("att0", att0), ("att1", att1), ("bout", bout)):
                sh = [128, 2, 4] if nm.startswith("att") else [128, 1]
                bt = constp.tile(sh, F32, tag=nm)
                nc.sync.dma_start(out=bt[:], in_=t[:])
                batt[nm] = bt
            bsb = {}
            for nm, t in (("bl0", bl0), ("br0", br0), ("bias0", bias0),
                          ("bl1", bl1), ("br1", br1), ("bias1", bias1)):
                bt = constp.tile([128, F], F32, tag=nm)
                nc.sync.dma_start(out=bt[:], in_=t[:])
                bsb[nm] = bt

            def load_w(t, kdim, tag):
                n = kdim // 128
                w = wpool.tile([128, n, F], F32R, tag=tag)
                for k in range(n):
                    nc.sync.dma_start(out=w[:, k, :], in_=t[k * 128:(k + 1) * 128, :])
                return w
            wl0_sb = load_w(wl0, cfg.IN_F, "wl0")
            wr0_sb = load_w(wr0, cfg.IN_F, "wr0")
            wl1_sb = load_w(wl1, F, "wl1")
            wr1_sb = load_w(wr1, F, "wr1")
            we0_sb = wpool.tile([EF, F], F32R, tag="we0")
            nc.sync.dma_start(out=we0_sb[:], in_=we0[:])
            we1_sb = wpool.tile([EF, F], F32R, tag="we1")
            nc.sync.dma_start(out=we1_sb[:], in_=we1[:])
            wout_sb = wpool.tile([128, 2, 1], F32, tag="wout")
            for h in range(2):
                nc.sync.dma_start(out=wout_sb[:, h, :], in_=wout[h * 128:(h + 1) * 128, :])

            # ---------------- P1: layer-0 projections ----------------
            for c in range(NW if "p1" in cfg.phases else 0):
                xk = io.tile([128, cfg.KCH, 128], F32R, tag="xk")
                for k in range(cfg.KCH):
                    nc.sync.dma_start(
                        out=xk[:, k, :],
                        in_=x_T[k * 128:(k + 1) * 128, c * 128:(c + 1) * 128])
                pl = psA.tile([128, F], F32, tag="pm")
                pr = psA.tile([128, F], F32, tag="pm")
                for k in range(cfg.KCH):
                    nc.tensor.matmul(pl[:], lhsT=xk[:, k, :], rhs=wl0_sb[:, k, :],
                                     start=(k == 0), stop=(k == cfg.KCH - 1),
                                     skip_group_check=True)
                for k in range(cfg.KCH):
                    nc.tensor.matmul(pr[:], lhsT=xk[:, k, :], rhs=wr0_sb[:, k, :],
                                     start=(k == 0), stop=(k == cfg.KCH - 1),
                                     skip_group_check=True)
                ol = io.tile([128, F], F32R, tag="oxl")
                orr = io.tile([128, F], F32R, tag="oxr")
                nc.vector.tensor_add(out=ol[:], in0=pl[:], in1=bsb["bl0"][:])
                nc.vector.tensor_add(out=orr[:], in0=pr[:], in1=bsb["br0"][:])
                nc.sync.dma_start(out=xl0_own[c * 128:(c + 1) * 128, :], in_=ol[:])
                nc.sync.dma_start(out=xr0_own[c * 128:(c + 1) * 128, :], in_=orr[:])

            if "ag0" in cfg.phases:
                tc.strict_bb_all_engine_barrier()
                nc.gpsimd.collective_compute(
                    "AllGather", mybir.AluOpType.bypass, replica_groups=groups,
                    ins=[xl0_own[:]], outs=[xl0_full[:]])
                tc.strict_bb_all_engine_barrier()

            # ---------------- edge pass (shared for both layers) ------------
            def edge_pass(layer, xl_full, xr_own, we_sb, att_sb, bias_sb):
                for w in range(NW):
                    xr_win = io.tile([128, F], F32R, tag="xrw")
                    nc.sync.dma_start(out=xr_win[:],
                                      in_=xr_own[w * W:(w + 1) * W, :])
                    pd = psW.tile([128, 4], F32, tag="wacc")
                    s_ts, st_ts, ex_es, stages = [], [], [], []
                    for j in range(J):
                        t = w * J + j
                        reg = nc.gpsimd.alloc_register()
                        nc.gpsimd.load(reg, klow_sb[0:1, t:t + 1])
                        stage = selp.tile([128, G, F], F32R, tag="stage")
                        iht = io.tile([128, T_E // 16], I16, tag="ih")
                        ilt = io.tile([128, T_E // 16], I16, tag="il")
                        nc.sync.dma_start(out=iht[:], in_=idx_hi[t])
                        nc.sync.dma_start(out=ilt[:], in_=idx_lo[t])
                        nc.gpsimd.dma_gather(
                            out_ap=stage[:], in_ap=xl_full[cfg.HALF:, :],
                            idxs_ap=iht[:], num_idxs=T_E, num_idxs_reg=T_E,
                            elem_size=F)
                        nc.gpsimd.dma_gather(
                            out_ap=stage[:], in_ap=xl_full[:cfg.HALF, :],
                            idxs_ap=ilt[:], num_idxs=T_E, num_idxs_reg=reg,
                            elem_size=F)
                        ea_t = io.tile([EF, T_E], F32R, tag="ea")
                        nc.sync.dma_start(out=ea_t[:], in_=eat[t])
                        dstf_t = io.tile([128, G], F32, tag="dstf")
                        nc.sync.dma_start(out=dstf_t[:], in_=dstf[t])
                        drow = io.tile([128, T_E], F32, tag="drow")
                        nc.sync.dma_start(
                            out=drow[:],
                            in_=dstrow[t:t + 1, :].to_broadcast([128, T_E]))
                        s_t = selp.tile([128, G, 128], F32R, tag="s")
                        for g in range(G):
                            nc.vector.tensor_tensor(
                                out=s_t[:, g, :],
                                in0=dstf_t[:, g:g + 1].to_broadcast([128, 128]),
                                in1=iota_row[:], op=mybir.AluOpType.is_equal)
                        st_t = selp.tile([128, T_E], F32R, tag="st")
                        nc.vector.tensor_tensor(
                            out=st_t[:], in0=iota_col[:].to_broadcast([128, T_E]),
                            in1=drow[:], op=mybir.AluOpType.is_equal)
                        pm = psA.tile([128, 2, T_E], F32, tag="pm")
                        for h in range(2):
                            nc.tensor.matmul(
                                pm[:, h, :], lhsT=we_sb[:, h * 128:(h + 1) * 128],
                                rhs=ea_t[:], start=True, stop=False,
                                skip_group_check=True)
                            nc.tensor.matmul(
                                pm[:, h, :],
                                lhsT=xr_win[:, h * 128:(h + 1) * 128],
                                rhs=st_t[:], start=False, stop=False,
                                skip_group_check=True)
                        for g in range(G):
                            for h in range(2):
                                nc.tensor.matmul(
                                    pm[:, h, g * 128:(g + 1) * 128],
                                    lhsT=stage[:, g, h * 128:(h + 1) * 128].bitcast(F32),
                                    rhs=ident[:], is_transpose=True,
                                    start=False, stop=(g == G - 1),
                                    skip_group_check=True)
                        m_t = mpool.tile([128, 2, T_E], F32, tag="m")
                        rp = mpool.tile([128, 2, T_E], F32, tag="rp")
                        for h in range(2):
                            nc.scalar.activation(
                                rp[:, h, :], pm[:, h, :],
                                mybir.ActivationFunctionType.Relu,
                                scale=1.0 - NEG_SLOPE)
                            nc.vector.scalar_tensor_tensor(
                                out=m_t[:, h, :], in0=pm[:, h, :],
                                scalar=NEG_SLOPE, in1=rp[:, h, :],
                                op0=mybir.AluOpType.mult,
                                op1=mybir.AluOpType.add)
                        plog = psS.tile([128, 16], F32, tag="sm")
                        for g in range(G):
                            for h in range(2):
                                nc.tensor.matmul(
                                    plog[:, g * 4:(g + 1) * 4],
                                    lhsT=m_t[:, h, g * 128:(g + 1) * 128],
                                    rhs=att_sb[:, h, :],
                                    start=(h == 0), stop=(h == 1),
                                    skip_group_check=True)
                        ex_e = selp.tile([128, 16], F32, tag="ex")
                        nc.scalar.activation(ex_e[:], plog[:],
                                             mybir.ActivationFunctionType.Exp)
                        for g in range(G):
                            nc.tensor.matmul(
                                pd[:], lhsT=s_t[:, g, :].bitcast(F32),
                                rhs=ex_e[:, g * 4:(g + 1) * 4],
                                start=(j == 0 and g == 0),
                                stop=(j == J - 1 and g == G - 1),
                                skip_group_check=True)
                        s_ts.append(s_t); st_ts.append(st_t); ex_es.append(ex_e); stages.append(stage)
                    rdf = small.tile([128, 4], F32, tag="rdf")
                    nc.vector.tensor_scalar_add(out=rdf[:], in0=pd[:], scalar1=EPS)
                    rden = small.tile([128, 4], F32, tag="rden")
                    nc.vector.reciprocal(out=rden[:], in_=rdf[:])
                    pagg = psW.tile([128, F], F32, tag="wacc")
                    for j in range(J):
                        s_t, st_t, ex_e = s_ts[j], st_ts[j], ex_es[j]
                        stage = stages[j]
                        pr = psS.tile([128, 16], F32, tag="sm")
                        for g in range(G):
                            nc.tensor.matmul(
                                pr[:, g * 4:(g + 1) * 4],
                                lhsT=st_t[:, g * 128:(g + 1) * 128].bitcast(F32),
                                rhs=rden[:], start=True, stop=True,
                                skip_group_check=True)
                        alpha = small.tile([128, 16], F32, tag="alpha")
                        nc.vector.tensor_tensor(out=alpha[:], in0=ex_e[:],
                                                in1=pr[:],
                                                op=mybir.AluOpType.mult)
                        for g in range(G):
                            v = small.tile([128, 4, 64], F32R, tag="v")
                            nc.vector.tensor_tensor(
                                out=v[:],
                                in0=stage[:, g, :].bitcast(F32)
                                    .rearrange("p (h c) -> p h c", h=4),
                                in1=alpha[:, g * 4:(g + 1) * 4].unsqueeze(-1)
                                    .to_broadcast([128, 4, 64]),
                                op=mybir.AluOpType.mult)
                            nc.tensor.matmul(
                                pagg[:], lhsT=s_t[:, g, :],
                                rhs=v[:].rearrange("p h c -> p (h c)"),
                                start=(j == 0 and g == 0),
                                stop=(j == J - 1 and g == G - 1),
                                skip_group_check=True)
                    hsum = small.tile([128, F], F32, tag="hsum")
                    nc.vector.tensor_add(out=hsum[:], in0=pagg[:], in1=bias_sb[:])
                    h_out = small.tile([128, F], F32, tag="hout")
                    nc.scalar.activation(h_out[:], hsum[:],
                                         mybir.ActivationFunctionType.Relu)
                    if layer == 0:
                        nc.sync.dma_start(out=h1_own[w * W:(w + 1) * W, :],
                                          in_=h_out[:])
                    else:
                        po = psW.tile([128, 1], F32, tag="wacc")
                        for h in range(2):
                            pt = psS.tile([128, 128], F32, tag="sm")
                            nc.tensor.matmul(pt[:], lhsT=h_out[:, h * 128:(h + 1) * 128],
                                             rhs=ident[:], is_transpose=True,
                                             start=True, stop=True,
                                             skip_group_check=True)
                            h2T = small.tile([128, 128], F32, tag="h2T")
                            nc.vector.tensor_copy(out=h2T[:], in_=pt[:])
                            nc.tensor.matmul(po[:], lhsT=h2T[:],
                                             rhs=wout_sb[:, h, :],
                                             start=(h == 0), stop=(h == 1),
                                             skip_group_check=True)
                        o_sb = small.tile([128, 1], F32, tag="osb")
                        nc.vector.tensor_scalar(
                            out=o_sb[:], in0=po[:], scalar1=batt["bout"][:, :1],
                            scalar2=None, op0=mybir.AluOpType.add)
                        nc.sync.dma_start(out=out_own[w * W:(w + 1) * W, :],
                                          in_=o_sb[:])

            if "e0" in cfg.phases:
                edge_pass(0, xl0_full, xr0_own, we0_sb, batt["att0"], bsb["bias0"])
            tc.strict_bb_all_engine_barrier()

            # ---------------- P4: layer-1 projections ----------------
            for c in range(NW if "p4" in cfg.phases else 0):
                h1c = io.tile([128, F], F32, tag="h1c")
                nc.sync.dma_start(out=h1c[:], in_=h1_own[c * 128:(c + 1) * 128, :])
                h1T = io.tile([128, 2, 128], F32R, tag="h1T")
                for h in range(2):
                    pt = psS.tile([128, 128], F32, tag="sm")
                    nc.tensor.matmul(pt[:], lhsT=h1c[:, h * 128:(h + 1) * 128],
                                     rhs=ident[:], is_transpose=True,
                                     start=True, stop=True, skip_group_check=True)
                    nc.vector.tensor_copy(out=h1T[:, h, :], in_=pt[:])
                pl = psA.tile([128, F], F32, tag="pm")
                pr = psA.tile([128, F], F32, tag="pm")
                for h in range(2):
                    nc.tensor.matmul(pl[:], lhsT=h1T[:, h, :], rhs=wl1_sb[:, h, :],
                                     start=(h == 0), stop=(h == 1),
                                     skip_group_check=True)
                for h in range(2):
                    nc.tensor.matmul(pr[:], lhsT=h1T[:, h, :], rhs=wr1_sb[:, h, :],
                                     start=(h == 0), stop=(h == 1),
                                     skip_group_check=True)
                ol = io.tile([128, F], F32R, tag="oxl")
                orr = io.tile([128, F], F32R, tag="oxr")
                nc.vector.tensor_add(out=ol[:], in0=pl[:], in1=bsb["bl1"][:])
                nc.vector.tensor_add(out=orr[:], in0=pr[:], in1=bsb["br1"][:])
                nc.sync.dma_start(out=xl1_own[c * 128:(c + 1) * 128, :], in_=ol[:])
                nc.sync.dma_start(out=xr1_own[c * 128:(c + 1) * 128, :], in_=orr[:])

            if "ag1" in cfg.phases:
                tc.strict_bb_all_engine_barrier()
                nc.gpsimd.collective_compute(
                    "AllGather", mybir.AluOpType.bypass, replica_groups=groups,
                    ins=[xl1_own[:]], outs=[xl1_full[:]])
                tc.strict_bb_all_engine_barrier()
            if "e1" in cfg.phases:
                edge_pass(1, xl1_full, xr1_own, we1_sb, batt["att1"], bsb["bias1"])
    return P


_CACHE = {}


def _get_compiled(cfg):
    key = (cfg.N, cfg.E, cfg.IN_F, cfg.NC, cfg.J)
    if key not in _CACHE:
        nc = bacc.Bacc("TRN2", target_bir_lowering=False, debug=False,
                       num_devices=cfg.NC)
        build_program(cfg, nc)
        nc.compile()
        _CACHE[key] = nc
    return _CACHE[key]


def make_in_maps(cfg, inputs, cores_pre):
    """Per-core input dicts."""
    x = np.asarray(inputs["x"], np.float32)
    H, C, F = cfg.H, cfg.C, cfg.F
    att_blk = {}
    for li in (0, 1):
        att = np.asarray(inputs[f"att{li}"], np.float32)   # [H, C]
        A = np.zeros((2 * 128, 4), np.float32)
        for h in range(H):
            A[h * C:(h + 1) * C, h] = att[h]
        att_blk[li] = np.ascontiguousarray(A.reshape(2, 128, 4).transpose(1, 0, 2))
    iota_r = np.tile(np.arange(128, dtype=np.float32)[None, :], (128, 1))
    iota_c = np.arange(128, dtype=np.float32).reshape(128, 1)
    rep = lambda v: np.tile(np.asarray(v, np.float32)[None, :], (128, 1))
    common = dict(
        wl0=f32r_round(inputs["W_l0"]), wr0=f32r_round(inputs["W_r0"]),
        we0=f32r_round(inputs["W_e0"]), wl1=f32r_round(inputs["W_l1"]),
        wr1=f32r_round(inputs["W_r1"]), we1=f32r_round(inputs["W_e1"]),
        wout=np.asarray(inputs["W_out"], np.float32),
        att0=att_blk[0], att1=att_blk[1],
        bl0=rep(inputs["b_l0"]), br0=rep(inputs["b_r0"]),
        bias0=rep(inputs["bias0"]), bl1=rep(inputs["b_l1"]),
        br1=rep(inputs["b_r1"]), bias1=rep(inputs["bias1"]),
        bout=np.tile(np.asarray(inputs["b_out"], np.float32).reshape(1, 1),
                     (128, 1)),
        iota_r=iota_r, iota_c=iota_c,
    )
    in_maps = []
    for c in range(cfg.NC):
        pre = cores_pre[c]
        xs = np.zeros((cfg.NVP, cfg.IN_F), np.float32)
        xs[:cfg.NV] = x[c * cfg.NV:(c + 1) * cfg.NV]
        m = dict(common)
        m.update(x_T=f32r_round(np.ascontiguousarray(xs.T)),
                 idx_hi=pre["idx_hi"], idx_lo=pre["idx_lo"],
                 klow=pre["klow"], dstf=pre["dstf"], dstrow=pre["dstrow"],
                 eat=pre["eat"])
        in_maps.append(m)
    return in_maps


def kernel(**inputs):
    cfg = Cfg(N=50000, E=800000, IN_F=512, NC=8, J=5)
    # bump J if some window overflows (keeps NEFF cache stable otherwise)
    dst = np.asarray(inputs["edge_index"][1], np.int64)
    loc = dst % cfg.NV
    wid = (dst // cfg.NV) * cfg.NW + loc // cfg.W
    need = np.bincount(wid, minlength=cfg.NC * cfg.NW).max()
    while need > cfg.J * (cfg.T_E - 1):
        cfg = Cfg(N=50000, E=800000, IN_F=512, NC=8, J=cfg.J + 1)
    cores_pre = preprocess(cfg, inputs["edge_index"], inputs["edge_attr"])
    in_maps = make_in_maps(cfg, inputs, cores_pre)
    nc = _get_compiled(cfg)
    res = run_bass_kernel_spmd(nc, in_maps, core_ids=list(range(cfg.NC)))
    outs = []
    for c in range(cfg.NC):
        outs.append(res.results[c]["out_own"][:cfg.NV])
    return np.concatenate(outs, 0).astype(np.float32)
